# revision 24
# baseline (speedup 1.0000x reference)
"""Distributed KNN-retrieval kernel for 8 Trainium2 NeuronCores.

Reference computation:
  protos = MLP(input_state)                 # [5, 20]
  dists[s, n] = || candidate_docs[n] - protos[s] ||_2
  top-100 smallest per slate -> (candidates [500, 20], indices [500])

Device strategy (SPMD over 8 cores, candidate_docs row-sharded):
  - Host pre-packs each 125k-row shard as fp16 [128, 20992] (rows 0-119 carry
    6 docs x 20 dims per column; rows 120-127 are zero pad -- full-128-partition
    DMA is ~2.2x faster than 120-partition DMA on this part).
  - On-device MLP (fp32 PE + scalar-engine leaky-relu) reproduces the reference
    policy net and emits the proto matrix transposed [20, 5].
  - Stationary A [120, 32]: 6 copies of protoT on the block diagonal.
    Stationary B [120, 32]: block-ones.  A matmul of A against a [120, 512]
    moving slice yields dot products for 6 docs x 5 slates; B against the
    squared docs yields the 6 doc norms.  Dots and norms go to SEPARATE psum
    banks so norm matmuls (no MLP dependency) keep the PE busy while the
    MLP/stationary-A chain resolves.
  - Scores s = ||c||^2 - 2 c.p are assembled on host (fp16 is plenty for
    candidate SELECTION: validated max local rank of a true top-100 member is
    ~20), then the final top-100 is recomputed exactly (same jnp ops as the
    reference) over a pooled candidate set, so the returned indices and rows
    match the reference bitwise.
"""

import sys

sys.path.insert(0, "/opt/trn_rl_repo")

import numpy as np

# ---- problem constants (hardcoded per harness contract) ----
D = 20
SLATE = 5
TOPK = 100
N_DOCS = 1_000_000
HID = 256
SLOPE = 0.01

NCORES = 8
SHARD = N_DOCS // NCORES          # 125000
PACK = 6                          # docs packed per column
PDIM = PACK * D                   # 120 data partitions (+8 pad)
NCOL = 20992                      # columns per core: 6*20992 = 125952 >= 125000
PAD_SHARD = PACK * NCOL           # 125952
CHUNK_COLS = [1024, 3072, 4096, 4096, 4096, 4096, 512]  # sums to NCOL
NCHUNK = len(CHUNK_COLS)
POOL_K = 2048                     # per-slate host pool size for exact rerank

# small weights blob A (f32, [21, WACOLS]): rows 0-19 = W1T cols 0-255 and
# input_state at col 256; row 20 = b1 cols 0-255 and the constant 1.0 at 256.
# Layer 1 is then a single K=21 matmul including the bias.
WACOLS = 257
# weights blob B (f32, [128, WBCOLS])
WB_W2 = 0          # + 256*k + c          (c < 256)
WB_W3 = 512        # + 100*k + 20*j + c   (c < 20)
WB_B2R = 712       # row 0, + 128*m + c
WB_B3R = 968       # row 0, + 20*j + c    (100 cols)
WB_ONE = 1068      # row 0: constant 1.0
WBCOLS = 1069

_CACHE = {}


def _build_nc():
    """Build the per-core Bass/Tile graph (identical on all 8 cores)."""
    from concourse import bacc, mybir
    import concourse.tile as tile

    f32 = mybir.dt.float32
    f16 = mybir.dt.float16
    Act = mybir.ActivationFunctionType
    Alu = mybir.AluOpType

    nc = bacc.Bacc("TRN2", target_bir_lowering=False)

    docsT = nc.declare_dram_parameter("docsT", [128, NCOL], f16, isOutput=False)
    wa_in = nc.declare_dram_parameter("wbloba", [D + 1, WACOLS], f32, isOutput=False)
    wb_in = nc.declare_dram_parameter("wblobb", [128, WBCOLS], f32, isOutput=False)
    ones_in = nc.declare_dram_parameter("blockones", [PDIM, PACK], f16, isOutput=False)
    out_d = nc.declare_dram_parameter(
        "out", [NCHUNK, 128, 2048], f16, isOutput=True
    )

    with tile.TileContext(nc) as tc:
        with (
            tc.tile_pool(name="const", bufs=1) as const,
            tc.tile_pool(name="mlp", bufs=1) as mlp,
            tc.tile_pool(name="docs", bufs=5) as dpool,
            tc.tile_pool(name="sq", bufs=5) as qpool,
            tc.tile_pool(name="stg", bufs=3) as gpool,
            tc.tile_pool(name="psd", bufs=4, space="PSUM") as psdpool,
            tc.tile_pool(name="psn", bufs=4, space="PSUM") as psnpool,
        ):
            # ---------------- setup ----------------
            wa = const.tile([D + 1, WACOLS], f32)
            nc.sync.dma_start(out=wa[:], in_=wa_in[:])
            wb = const.tile([128, WBCOLS], f32)
            nc.scalar.dma_start(out=wb[:], in_=wb_in[:])

            # stationary [120, 64]: cols 0-29 proto block-diagonal (30-31 zero),
            # cols 32-37 block-ones (38-63 zero); both matmul stationaries are
            # 32 wide so full 32-partition quarters of PSUM get written.
            stat = const.tile([PDIM, 64], f16)
            nc.vector.memset(stat[:], 0.0)
            nc.scalar.dma_start(out=stat[:, 32 : 32 + PACK], in_=ones_in[:])

            def leaky(dst, src):
                # leaky(x) = (1+s)/2 * x + (1-s)/2 * |x| -- ACT (which can read
                # PSUM) + a gpsimd add; keeps the latency-critical MLP chain off
                # the DVE queue, which is busy with 2.2us doc-square streams.
                z = mlp.tile(list(src.shape), f32, tag="lkz", name="lkz")
                t = mlp.tile(list(src.shape), f32, tag="lkt", name="lkt")
                nc.scalar.activation(z[:], src, Act.Copy, scale=(1 + SLOPE) / 2)
                nc.scalar.activation(t[:], src, Act.Abs, scale=(1 - SLOPE) / 2)
                nc.gpsimd.tensor_tensor(out=dst, in0=z[:], in1=t[:], op=Alu.add)

            # ---------------- MLP (fp32) ----------------
            # layer 1+2 activations as [128, 2] (col m = units 128m..128m+127)
            ps1 = psdpool.tile([128, 2], f32, tag="psd")
            for m in range(2):
                nc.tensor.matmul(
                    out=ps1[:, m : m + 1],
                    lhsT=wa[0 : D + 1, 128 * m : 128 * (m + 1)],
                    rhs=wa[0 : D + 1, HID : HID + 1],
                    start=True,
                    stop=True,
                )
            h1 = mlp.tile([128, 2], f32, tag="h1")
            leaky(h1[:], ps1[:])

            ps2 = psdpool.tile([128, 2], f32, tag="psd")
            for m in range(2):
                for k in range(2):
                    nc.tensor.matmul(
                        out=ps2[:, m : m + 1],
                        lhsT=wb[:, WB_W2 + 256 * k + 128 * m : WB_W2 + 256 * k + 128 * (m + 1)],
                        rhs=h1[:, k : k + 1],
                        start=(k == 0),
                        stop=False,
                    )
                nc.tensor.matmul(
                    out=ps2[:, m : m + 1],
                    lhsT=wb[0:1, WB_B2R + 128 * m : WB_B2R + 128 * (m + 1)],
                    rhs=wb[0:1, WB_ONE : WB_ONE + 1],
                    start=False,
                    stop=True,
                )
            h2 = mlp.tile([128, 2], f32, tag="h2")
            leaky(h2[:], ps2[:])

            # layer 3: emit protoT [20, 5] directly (column j = proto_j)
            ps3 = psdpool.tile([D, SLATE], f32, tag="psd")
            for j in range(SLATE):
                for k in range(2):
                    nc.tensor.matmul(
                        out=ps3[:, j : j + 1],
                        lhsT=wb[:, WB_W3 + 100 * k + D * j : WB_W3 + 100 * k + D * (j + 1)],
                        rhs=h2[:, k : k + 1],
                        start=(k == 0),
                        stop=False,
                    )
                nc.tensor.matmul(
                    out=ps3[:, j : j + 1],
                    lhsT=wb[0:1, WB_B3R + D * j : WB_B3R + D * (j + 1)],
                    rhs=wb[0:1, WB_ONE : WB_ONE + 1],
                    start=False,
                    stop=True,
                )
            pt32 = mlp.tile([D, SLATE], f32, tag="pt32")
            leaky(pt32[:], ps3[:])
            pt16 = mlp.tile([D, SLATE], f16, tag="pt16")
            nc.gpsimd.tensor_copy(out=pt16[:], in_=pt32[:])

            # place 6 copies of protoT on the block diagonal (2 engines)
            for g in range(PACK):
                eng = nc.scalar if g % 2 == 0 else nc.gpsimd
                eng.dma_start(
                    out=stat[D * g : D * (g + 1), SLATE * g : SLATE * (g + 1)],
                    in_=pt16[:],
                )

            # ---------------- main loop over doc chunks ----------------
            col0 = 0
            for c, cols in enumerate(CHUNK_COLS):
                nslice = cols // 512
                nhalf = (nslice + 3) // 4          # psum tiles per kind
                dck = dpool.tile([128, cols], f16, tag="d", name=f"d{c}")
                nc.sync.dma_start(out=dck[:], in_=docsT[:, col0 : col0 + cols])
                sck = qpool.tile([PDIM, cols], f16, tag="q", name=f"q{c}")
                nc.vector.tensor_tensor(
                    out=sck[:], in0=dck[0:PDIM, :], in1=dck[0:PDIM, :], op=Alu.mult
                )

                srows = 128 if nslice >= 4 else 32 * nslice
                stg = gpool.tile([srows, 1024 * nhalf], f16, tag="s", name=f"s{c}")
                psD = [
                    psdpool.tile([srows, 512], f32, tag="psd", name=f"psD{c}_{h}")
                    for h in range(nhalf)
                ]
                psN = [
                    psnpool.tile([srows, 512], f32, tag="psn", name=f"psN{c}_{h}")
                    for h in range(nhalf)
                ]
                for s in range(nslice):
                    half, q = s // 4, s % 4
                    mv = slice(512 * s, 512 * (s + 1))
                    nc.tensor.matmul(
                        out=psD[half][32 * q : 32 * q + 32, :],
                        lhsT=stat[:, 0:32],
                        rhs=dck[0:PDIM, mv],
                        start=True,
                        stop=True,
                        tile_position=(0, 32 * q),
                    )
                    nc.tensor.matmul(
                        out=psN[half][32 * q : 32 * q + 32, :],
                        lhsT=stat[:, 32:64],
                        rhs=sck[:, mv],
                        start=True,
                        stop=True,
                        tile_position=(0, 32 * q),
                    )
                # staging layout: [dots halves... | norms halves...]
                for h in range(nhalf):
                    nc.scalar.activation(
                        stg[:, 512 * h : 512 * (h + 1)], psD[h][:], Act.Copy
                    )
                    nc.vector.tensor_copy(
                        out=stg[:, 512 * (nhalf + h) : 512 * (nhalf + h + 1)],
                        in_=psN[h][:],
                    )
                nc.gpsimd.dma_start(
                    out=out_d[c, 0:srows, 0 : 1024 * nhalf], in_=stg[:]
                )
                col0 += cols

    return nc


def _get_nc():
    if "nc" not in _CACHE:
        nc = _build_nc()
        nc.finalize()  # Bacc: reg-alloc + codegen passes + freeze
        _CACHE["nc"] = nc
    return _CACHE["nc"]


def make_in_maps(input_state, candidate_docs, W1, b1, W2, b2, W3, b3):
    """Shard + lay out host inputs for the 8 cores."""
    blockones = np.zeros((PDIM, PACK), np.float16)
    for g in range(PACK):
        blockones[D * g : D * (g + 1), g] = 1.0
    wb = np.zeros((128, WBCOLS), np.float32)
    W1, W2, W3 = (np.asarray(a, np.float32) for a in (W1, W2, W3))
    b1, b2, b3 = (np.asarray(a, np.float32) for a in (b1, b2, b3))
    for k in range(2):
        wb[:, WB_W2 + 256 * k : WB_W2 + 256 * (k + 1)] = W2.T[128 * k : 128 * (k + 1), :]
        wb[:, WB_W3 + 100 * k : WB_W3 + 100 * (k + 1)] = W3.T[128 * k : 128 * (k + 1), :]
    wb[0, WB_B2R : WB_B2R + HID] = b2
    wb[0, WB_B3R : WB_B3R + SLATE * D] = b3
    wb[0, WB_ONE] = 1.0
    wa = np.zeros((D + 1, WACOLS), np.float32)
    wa[0:D, 0:HID] = W1.T
    wa[0:D, HID] = np.asarray(input_state, np.float32)
    wa[D, 0:HID] = b1
    wa[D, HID] = 1.0
    in_maps = []
    for i in range(NCORES):
        shard = candidate_docs[i * SHARD : (i + 1) * SHARD]
        pad = np.zeros((PAD_SHARD, D), np.float16)
        pad[:SHARD] = shard.astype(np.float16)
        docsT = np.zeros((128, NCOL), np.float16)
        docsT[:PDIM] = pad.reshape(NCOL, PDIM).T
        in_maps.append(
            {"docsT": docsT, "wbloba": wa, "wblobb": wb, "blockones": blockones}
        )
    return in_maps


def decode_scores(out_arr):
    """[NCHUNK, 128, CHUNK//2] fp16 device output -> per-core scores [SHARD, 5]."""
    dots = np.empty((NCOL, PACK, SLATE), np.float32)
    norms = np.empty((NCOL, PACK), np.float32)
    col0 = 0
    for c, cols in enumerate(CHUNK_COLS):
        nslice = cols // 512
        nhalf = (nslice + 3) // 4
        blk = out_arr[c].astype(np.float32)
        for s in range(nslice):
            half, q = s // 4, s % 4
            n0 = col0 + 512 * s
            dblk = blk[32 * q : 32 * q + 30, 512 * half : 512 * (half + 1)]
            nblk = blk[32 * q : 32 * q + 6, 512 * (nhalf + half) : 512 * (nhalf + half + 1)]
            dots[n0 : n0 + 512] = dblk.reshape(PACK, SLATE, 512).transpose(2, 0, 1)
            norms[n0 : n0 + 512] = nblk.T
        col0 += cols
    scores = norms[:, :, None] - 2.0 * dots
    return scores.reshape(PAD_SHARD, SLATE)[:SHARD]


def run_device(inputs):
    """Run the 8-core kernel; returns (scores [N_DOCS, 5] fp32, results)."""
    from concourse.bass_utils import run_bass_kernel_spmd

    nc = _get_nc()
    in_maps = make_in_maps(**inputs)
    res = run_bass_kernel_spmd(nc, in_maps, core_ids=list(range(NCORES)))
    parts = [decode_scores(np.asarray(res.results[i]["out"])) for i in range(NCORES)]
    return np.concatenate(parts, axis=0), res


def finalize(scores, input_state, candidate_docs, W1, b1, W2, b2, W3, b3):
    """Host pool + exact rerank replicating the reference ops bitwise."""
    import jax
    import jax.numpy as jnp

    pool = []
    for j in range(SLATE):
        pool.append(np.argpartition(scores[:, j], POOL_K)[:POOL_K])
    pool = np.unique(np.concatenate(pool))  # sorted ascending
    cpu = jax.devices("cpu")[0]
    with jax.default_device(cpu):
        x = jnp.asarray(input_state)
        for W, b in ((W1, b1), (W2, b2), (W3, b3)):
            x = jax.nn.leaky_relu(x @ jnp.asarray(W).T + jnp.asarray(b),
                                  negative_slope=SLOPE)
        proto_slate = x.reshape(SLATE, D)
        docs_pool = jnp.asarray(candidate_docs[pool])
        diff = docs_pool[None, :, :] - proto_slate[:, None, :]
        dists = jnp.sqrt(jnp.sum(diff * diff, axis=-1))
        _, idx = jax.lax.top_k(-dists, TOPK)
        idx = np.asarray(idx)
    indices = pool[idx].reshape(-1).astype(np.int32)
    candidates = candidate_docs[indices]
    return candidates, indices


def kernel(**inputs):
    inputs = {k: np.asarray(v) for k, v in inputs.items()}
    scores, _ = run_device(inputs)
    return finalize(scores, **inputs)


# revision 25
# speedup vs baseline: 1.1966x; 1.1966x over previous
"""Distributed KNN-retrieval kernel for 8 Trainium2 NeuronCores.

Reference computation:
  protos = MLP(input_state)                 # [5, 20]
  dists[s, n] = || candidate_docs[n] - protos[s] ||_2
  top-100 smallest per slate -> (candidates [500, 20], indices [500])

Device strategy (SPMD over 8 cores, candidate_docs row-sharded):
  - Host pre-packs each 125k-row shard as fp16 [128, 20992] (rows 0-119 carry
    6 docs x 20 dims per column; rows 120-127 are zero pad -- full-128-partition
    DMA is ~2.2x faster than 120-partition DMA on this part).
  - The tiny policy net (151KB of weights -> 100 outputs, ~0.1% of the FLOPs)
    is replicated per query on the host, exactly as the reference defines it;
    attempts at running it on-device put ~12-15us of pure semaphore/DMA
    latency (10 serial engine hops) on the critical path for 0.1% of the work.
  - Stationary A [120, 32]: 6 copies of protoT on the block diagonal.
    Stationary B [120, 32]: block-ones.  A matmul of A against a [120, 512]
    moving slice yields dot products for 6 docs x 5 slates; B against the
    squared docs yields the 6 doc norms.  Dots and norms go to SEPARATE psum
    banks so norm matmuls (no MLP dependency) keep the PE busy while the
    MLP/stationary-A chain resolves.
  - Scores s = ||c||^2 - 2 c.p are assembled on host (fp16 is plenty for
    candidate SELECTION: validated max local rank of a true top-100 member is
    ~20), then the final top-100 is recomputed exactly (same jnp ops as the
    reference) over a pooled candidate set, so the returned indices and rows
    match the reference bitwise.
"""

import sys

sys.path.insert(0, "/opt/trn_rl_repo")

import numpy as np

# ---- problem constants (hardcoded per harness contract) ----
D = 20
SLATE = 5
TOPK = 100
N_DOCS = 1_000_000
HID = 256
SLOPE = 0.01

NCORES = 8
SHARD = N_DOCS // NCORES          # 125000
PACK = 6                          # docs packed per column
PDIM = PACK * D                   # 120 data partitions (+8 pad)
NCOL = 20992                      # columns per core: 6*20992 = 125952 >= 125000
PAD_SHARD = PACK * NCOL           # 125952
CHUNK_COLS = [1024, 3072, 4096, 4096, 4096, 4096, 512]  # sums to NCOL
NCHUNK = len(CHUNK_COLS)
POOL_K = 2048                     # per-slate host pool size for exact rerank

# small weights blob A (f32, [21, WACOLS]): rows 0-19 = W1T cols 0-255 and
# input_state at col 256; row 20 = b1 cols 0-255 and the constant 1.0 at 256.
# Layer 1 is then a single K=21 matmul including the bias.
WACOLS = 257
# weights blob B (f32, [128, WBCOLS])
WB_W2 = 0          # + 256*k + c          (c < 256)
WB_W3 = 512        # + 100*k + 20*j + c   (c < 20)
WB_B2R = 712       # row 0, + 128*m + c
WB_B3R = 968       # row 0, + 20*j + c    (100 cols)
WB_ONE = 1068      # row 0: constant 1.0
WBCOLS = 1069

_CACHE = {}


def _build_nc():
    """Build the per-core Bass/Tile graph (identical on all 8 cores)."""
    from concourse import bacc, mybir
    import concourse.tile as tile

    f32 = mybir.dt.float32
    f16 = mybir.dt.float16
    Act = mybir.ActivationFunctionType
    Alu = mybir.AluOpType

    nc = bacc.Bacc("TRN2", target_bir_lowering=False)

    docsT = nc.declare_dram_parameter("docsT", [128, NCOL], f16, isOutput=False)
    stat_in = nc.declare_dram_parameter("statin", [PDIM, 64], f16, isOutput=False)
    out_d = nc.declare_dram_parameter(
        "out", [NCHUNK, 128, 2048], f16, isOutput=True
    )

    with tile.TileContext(nc) as tc:
        with (
            tc.tile_pool(name="const", bufs=1) as const,
            tc.tile_pool(name="docs", bufs=5) as dpool,
            tc.tile_pool(name="sq", bufs=5) as qpool,
            tc.tile_pool(name="stg", bufs=3) as gpool,
            tc.tile_pool(name="psd", bufs=4, space="PSUM") as psdpool,
            tc.tile_pool(name="psn", bufs=4, space="PSUM") as psnpool,
        ):
            # ---------------- setup ----------------
            # stationary [120, 64]: cols 0-29 proto block-diagonal (30-31 zero),
            # cols 32-37 block-ones (38-63 zero); both matmul stationaries are
            # 32 wide so full 32-partition quarters of PSUM get written.
            stat = const.tile([PDIM, 64], f16)
            nc.sync.dma_start(out=stat[:], in_=stat_in[:])

            # ---------------- main loop over doc chunks ----------------
            col0 = 0
            for c, cols in enumerate(CHUNK_COLS):
                nslice = cols // 512
                nhalf = (nslice + 3) // 4          # psum tiles per kind
                dck = dpool.tile([128, cols], f16, tag="d", name=f"d{c}")
                nc.sync.dma_start(out=dck[:], in_=docsT[:, col0 : col0 + cols])
                sck = qpool.tile([PDIM, cols], f16, tag="q", name=f"q{c}")
                nc.vector.tensor_tensor(
                    out=sck[:], in0=dck[0:PDIM, :], in1=dck[0:PDIM, :], op=Alu.mult
                )

                srows = 128 if nslice >= 4 else 32 * nslice
                stg = gpool.tile([srows, 1024 * nhalf], f16, tag="s", name=f"s{c}")
                psD = [
                    psdpool.tile([srows, 512], f32, tag="psd", name=f"psD{c}_{h}")
                    for h in range(nhalf)
                ]
                psN = [
                    psnpool.tile([srows, 512], f32, tag="psn", name=f"psN{c}_{h}")
                    for h in range(nhalf)
                ]
                for s in range(nslice):
                    half, q = s // 4, s % 4
                    mv = slice(512 * s, 512 * (s + 1))
                    nc.tensor.matmul(
                        out=psD[half][32 * q : 32 * q + 32, :],
                        lhsT=stat[:, 0:32],
                        rhs=dck[0:PDIM, mv],
                        start=True,
                        stop=True,
                        tile_position=(0, 32 * q),
                    )
                    nc.tensor.matmul(
                        out=psN[half][32 * q : 32 * q + 32, :],
                        lhsT=stat[:, 32:64],
                        rhs=sck[:, mv],
                        start=True,
                        stop=True,
                        tile_position=(0, 32 * q),
                    )
                # staging layout: [dots halves... | norms halves...]
                for h in range(nhalf):
                    nc.scalar.activation(
                        stg[:, 512 * h : 512 * (h + 1)], psD[h][:], Act.Copy
                    )
                    nc.scalar.activation(
                        stg[:, 512 * (nhalf + h) : 512 * (nhalf + h + 1)],
                        psN[h][:],
                        Act.Copy,
                    )
                nc.gpsimd.dma_start(
                    out=out_d[c, 0:srows, 0 : 1024 * nhalf], in_=stg[:]
                )
                col0 += cols

    return nc


def _get_nc():
    if "nc" not in _CACHE:
        nc = _build_nc()
        nc.finalize()  # Bacc: reg-alloc + codegen passes + freeze
        _CACHE["nc"] = nc
    return _CACHE["nc"]


def make_in_maps(input_state, candidate_docs, W1, b1, W2, b2, W3, b3):
    """Shard + lay out host inputs for the 8 cores."""
    x = np.asarray(input_state, np.float32)
    for W, b in ((W1, b1), (W2, b2), (W3, b3)):
        z = x @ np.asarray(W, np.float32).T + np.asarray(b, np.float32)
        x = np.where(z > 0, z, SLOPE * z)
    protoT = np.ascontiguousarray(x.reshape(SLATE, D).T).astype(np.float16)
    stat = np.zeros((PDIM, 64), np.float16)
    for g in range(PACK):
        stat[D * g : D * (g + 1), SLATE * g : SLATE * (g + 1)] = protoT
        stat[D * g : D * (g + 1), 32 + g] = 1.0
    in_maps = []
    for i in range(NCORES):
        shard = candidate_docs[i * SHARD : (i + 1) * SHARD]
        pad = np.zeros((PAD_SHARD, D), np.float16)
        pad[:SHARD] = shard.astype(np.float16)
        docsT = np.zeros((128, NCOL), np.float16)
        docsT[:PDIM] = pad.reshape(NCOL, PDIM).T
        in_maps.append({"docsT": docsT, "statin": stat})
    return in_maps


def decode_scores(out_arr):
    """[NCHUNK, 128, CHUNK//2] fp16 device output -> per-core scores [SHARD, 5]."""
    dots = np.empty((NCOL, PACK, SLATE), np.float32)
    norms = np.empty((NCOL, PACK), np.float32)
    col0 = 0
    for c, cols in enumerate(CHUNK_COLS):
        nslice = cols // 512
        nhalf = (nslice + 3) // 4
        blk = out_arr[c].astype(np.float32)
        for s in range(nslice):
            half, q = s // 4, s % 4
            n0 = col0 + 512 * s
            dblk = blk[32 * q : 32 * q + 30, 512 * half : 512 * (half + 1)]
            nblk = blk[32 * q : 32 * q + 6, 512 * (nhalf + half) : 512 * (nhalf + half + 1)]
            dots[n0 : n0 + 512] = dblk.reshape(PACK, SLATE, 512).transpose(2, 0, 1)
            norms[n0 : n0 + 512] = nblk.T
        col0 += cols
    scores = norms[:, :, None] - 2.0 * dots
    return scores.reshape(PAD_SHARD, SLATE)[:SHARD]


def run_device(inputs):
    """Run the 8-core kernel; returns (scores [N_DOCS, 5] fp32, results)."""
    from concourse.bass_utils import run_bass_kernel_spmd

    nc = _get_nc()
    in_maps = make_in_maps(**inputs)
    res = run_bass_kernel_spmd(nc, in_maps, core_ids=list(range(NCORES)))
    parts = [decode_scores(np.asarray(res.results[i]["out"])) for i in range(NCORES)]
    return np.concatenate(parts, axis=0), res


def finalize(scores, input_state, candidate_docs, W1, b1, W2, b2, W3, b3):
    """Host pool + exact rerank replicating the reference ops bitwise."""
    import jax
    import jax.numpy as jnp

    pool = []
    for j in range(SLATE):
        pool.append(np.argpartition(scores[:, j], POOL_K)[:POOL_K])
    pool = np.unique(np.concatenate(pool))  # sorted ascending
    cpu = jax.devices("cpu")[0]
    with jax.default_device(cpu):
        x = jnp.asarray(input_state)
        for W, b in ((W1, b1), (W2, b2), (W3, b3)):
            x = jax.nn.leaky_relu(x @ jnp.asarray(W).T + jnp.asarray(b),
                                  negative_slope=SLOPE)
        proto_slate = x.reshape(SLATE, D)
        docs_pool = jnp.asarray(candidate_docs[pool])
        diff = docs_pool[None, :, :] - proto_slate[:, None, :]
        dists = jnp.sqrt(jnp.sum(diff * diff, axis=-1))
        _, idx = jax.lax.top_k(-dists, TOPK)
        idx = np.asarray(idx)
    indices = pool[idx].reshape(-1).astype(np.int32)
    candidates = candidate_docs[indices]
    return candidates, indices


def kernel(**inputs):
    inputs = {k: np.asarray(v) for k, v in inputs.items()}
    scores, _ = run_device(inputs)
    return finalize(scores, **inputs)


# revision 27
# speedup vs baseline: 1.3417x; 1.1213x over previous
"""Distributed KNN-retrieval kernel for 8 Trainium2 NeuronCores.

Reference computation:
  protos = MLP(input_state)                 # [5, 20]
  dists[s, n] = || candidate_docs[n] - protos[s] ||_2
  top-100 smallest per slate -> (candidates [500, 20], indices [500])

Device strategy (SPMD over 8 cores, candidate_docs row-sharded):
  - Host pre-packs each 125k-row shard as fp16 [128, 20992] (rows 0-119 carry
    6 docs x 20 dims per column; rows 120-127 are zero pad -- full-128-partition
    DMA is ~2.2x faster than 120-partition DMA on this part).
  - The tiny policy net (151KB of weights -> 100 outputs, ~0.1% of the FLOPs)
    is replicated per query on the host, exactly as the reference defines it;
    attempts at running it on-device put ~12-15us of pure semaphore/DMA
    latency (10 serial engine hops) on the critical path for 0.1% of the work.
  - Stationary A [120, 32]: 6 copies of protoT on the block diagonal.
    Stationary B [120, 32]: block-ones.  A matmul of A against a [120, 512]
    moving slice yields dot products for 6 docs x 5 slates; B against the
    squared docs yields the 6 doc norms.  Dots and norms go to SEPARATE psum
    banks so norm matmuls (no MLP dependency) keep the PE busy while the
    MLP/stationary-A chain resolves.
  - Scores s = ||c||^2 - 2 c.p are assembled on host (fp16 is plenty for
    candidate SELECTION: validated max local rank of a true top-100 member is
    ~20), then the final top-100 is recomputed exactly (same jnp ops as the
    reference) over a pooled candidate set, so the returned indices and rows
    match the reference bitwise.
"""

import sys

sys.path.insert(0, "/opt/trn_rl_repo")

import numpy as np

# ---- problem constants (hardcoded per harness contract) ----
D = 20
SLATE = 5
TOPK = 100
N_DOCS = 1_000_000
HID = 256
SLOPE = 0.01

NCORES = 8
SHARD = N_DOCS // NCORES          # 125000
PACK = 6                          # docs packed per column
PDIM = PACK * D                   # 120 data partitions (+8 pad)
NCOL = 20992                      # columns per core: 6*20992 = 125952 >= 125000
PAD_SHARD = PACK * NCOL           # 125952
CHUNK_COLS = [2048] * 10 + [512]  # sums to NCOL
NCHUNK = len(CHUNK_COLS)
POOL_K = 2048                     # per-slate host pool size for exact rerank

# small weights blob A (f32, [21, WACOLS]): rows 0-19 = W1T cols 0-255 and
# input_state at col 256; row 20 = b1 cols 0-255 and the constant 1.0 at 256.
# Layer 1 is then a single K=21 matmul including the bias.
WACOLS = 257
# weights blob B (f32, [128, WBCOLS])
WB_W2 = 0          # + 256*k + c          (c < 256)
WB_W3 = 512        # + 100*k + 20*j + c   (c < 20)
WB_B2R = 712       # row 0, + 128*m + c
WB_B3R = 968       # row 0, + 20*j + c    (100 cols)
WB_ONE = 1068      # row 0: constant 1.0
WBCOLS = 1069

_CACHE = {}


def _build_nc():
    """Build the per-core Bass/Tile graph (identical on all 8 cores)."""
    from concourse import bacc, mybir
    import concourse.tile as tile

    f32 = mybir.dt.float32
    f16 = mybir.dt.float16
    Act = mybir.ActivationFunctionType
    Alu = mybir.AluOpType

    nc = bacc.Bacc("TRN2", target_bir_lowering=False)

    docsT = nc.declare_dram_parameter("docsT", [128, NCOL], f16, isOutput=False)
    stat_in = nc.declare_dram_parameter("statin", [PDIM, 64], f16, isOutput=False)
    out_d = nc.declare_dram_parameter(
        "out", [NCHUNK, 128, 1024], f16, isOutput=True
    )

    with tile.TileContext(nc) as tc:
        with (
            tc.tile_pool(name="const", bufs=1) as const,
            tc.tile_pool(name="docs", bufs=5) as dpool,
            tc.tile_pool(name="sq", bufs=5) as qpool,
            tc.tile_pool(name="stg", bufs=4) as gpool,
            tc.tile_pool(name="ps", bufs=4, space="PSUM") as pspool,
        ):
            # ---------------- setup ----------------
            # stationary [120, 64]: cols 0-29 proto block-diagonal (30-31 zero),
            # cols 32-37 block-ones (38-63 zero); both matmul stationaries are
            # 32 wide so full 32-partition quarters of PSUM get written.
            stat = const.tile([PDIM, 64], f16)
            nc.sync.dma_start(out=stat[:], in_=stat_in[:])

            # ---------------- main loop over doc chunks ----------------
            col0 = 0
            for c, cols in enumerate(CHUNK_COLS):
                nslice = cols // 512
                assert nslice <= 4
                dck = dpool.tile([128, cols], f16, tag="d", name=f"d{c}")
                nc.sync.dma_start(out=dck[:], in_=docsT[:, col0 : col0 + cols])
                sck = qpool.tile([PDIM, cols], f16, tag="q", name=f"q{c}")
                nc.vector.tensor_tensor(
                    out=sck[:], in0=dck[0:PDIM, :], in1=dck[0:PDIM, :], op=Alu.mult
                )

                srows = 32 * nslice
                # psum tile: bank 0 = dots (slice s at partition quarter s),
                # bank 1 = norms
                ps = pspool.tile([srows, 1024], f32, tag="ps", name=f"ps{c}")
                for s in range(nslice):
                    mv = slice(512 * s, 512 * (s + 1))
                    nc.tensor.matmul(
                        out=ps[32 * s : 32 * s + 32, 0:512],
                        lhsT=stat[:, 0:32],
                        rhs=dck[0:PDIM, mv],
                        start=True,
                        stop=True,
                        tile_position=(0, 32 * s),
                    )
                    nc.tensor.matmul(
                        out=ps[32 * s : 32 * s + 32, 512:1024],
                        lhsT=stat[:, 32:64],
                        rhs=sck[:, mv],
                        start=True,
                        stop=True,
                        tile_position=(0, 32 * s),
                    )
                stg = gpool.tile([srows, 1024], f16, tag="s", name=f"s{c}")
                nc.scalar.activation(stg[:], ps[:], Act.Copy)
                nc.gpsimd.dma_start(out=out_d[c, 0:srows, :], in_=stg[:])
                col0 += cols

    return nc


def _get_nc():
    if "nc" not in _CACHE:
        nc = _build_nc()
        nc.finalize()  # Bacc: reg-alloc + codegen passes + freeze
        _CACHE["nc"] = nc
    return _CACHE["nc"]


def make_in_maps(input_state, candidate_docs, W1, b1, W2, b2, W3, b3):
    """Shard + lay out host inputs for the 8 cores."""
    x = np.asarray(input_state, np.float32)
    for W, b in ((W1, b1), (W2, b2), (W3, b3)):
        z = x @ np.asarray(W, np.float32).T + np.asarray(b, np.float32)
        x = np.where(z > 0, z, SLOPE * z)
    protoT = np.ascontiguousarray(x.reshape(SLATE, D).T).astype(np.float16)
    stat = np.zeros((PDIM, 64), np.float16)
    for g in range(PACK):
        stat[D * g : D * (g + 1), SLATE * g : SLATE * (g + 1)] = protoT
        stat[D * g : D * (g + 1), 32 + g] = 1.0
    in_maps = []
    for i in range(NCORES):
        shard = candidate_docs[i * SHARD : (i + 1) * SHARD]
        pad = np.zeros((PAD_SHARD, D), np.float16)
        pad[:SHARD] = shard.astype(np.float16)
        docsT = np.zeros((128, NCOL), np.float16)
        docsT[:PDIM] = pad.reshape(NCOL, PDIM).T
        in_maps.append({"docsT": docsT, "statin": stat})
    return in_maps


def decode_scores(out_arr):
    """[NCHUNK, 128, 1024] fp16 device output -> per-core scores [SHARD, 5]."""
    dots = np.empty((NCOL, PACK, SLATE), np.float32)
    norms = np.empty((NCOL, PACK), np.float32)
    col0 = 0
    for c, cols in enumerate(CHUNK_COLS):
        nslice = cols // 512
        blk = out_arr[c].astype(np.float32)
        for s in range(nslice):
            n0 = col0 + 512 * s
            dblk = blk[32 * s : 32 * s + 30, 0:512]
            nblk = blk[32 * s : 32 * s + 6, 512:1024]
            dots[n0 : n0 + 512] = dblk.reshape(PACK, SLATE, 512).transpose(2, 0, 1)
            norms[n0 : n0 + 512] = nblk.T
        col0 += cols
    scores = norms[:, :, None] - 2.0 * dots
    return scores.reshape(PAD_SHARD, SLATE)[:SHARD]


def run_device(inputs):
    """Run the 8-core kernel; returns (scores [N_DOCS, 5] fp32, results)."""
    from concourse.bass_utils import run_bass_kernel_spmd

    nc = _get_nc()
    in_maps = make_in_maps(**inputs)
    res = run_bass_kernel_spmd(nc, in_maps, core_ids=list(range(NCORES)))
    parts = [decode_scores(np.asarray(res.results[i]["out"])) for i in range(NCORES)]
    return np.concatenate(parts, axis=0), res


def finalize(scores, input_state, candidate_docs, W1, b1, W2, b2, W3, b3):
    """Host pool + exact rerank replicating the reference ops bitwise."""
    import jax
    import jax.numpy as jnp

    pool = []
    for j in range(SLATE):
        pool.append(np.argpartition(scores[:, j], POOL_K)[:POOL_K])
    pool = np.unique(np.concatenate(pool))  # sorted ascending
    cpu = jax.devices("cpu")[0]
    with jax.default_device(cpu):
        x = jnp.asarray(input_state)
        for W, b in ((W1, b1), (W2, b2), (W3, b3)):
            x = jax.nn.leaky_relu(x @ jnp.asarray(W).T + jnp.asarray(b),
                                  negative_slope=SLOPE)
        proto_slate = x.reshape(SLATE, D)
        docs_pool = jnp.asarray(candidate_docs[pool])
        diff = docs_pool[None, :, :] - proto_slate[:, None, :]
        dists = jnp.sqrt(jnp.sum(diff * diff, axis=-1))
        _, idx = jax.lax.top_k(-dists, TOPK)
        idx = np.asarray(idx)
    indices = pool[idx].reshape(-1).astype(np.int32)
    candidates = candidate_docs[indices]
    return candidates, indices


def kernel(**inputs):
    inputs = {k: np.asarray(v) for k, v in inputs.items()}
    scores, _ = run_device(inputs)
    return finalize(scores, **inputs)
